# revision 15
# baseline (speedup 1.0000x reference)
"""nn_Classifier kernel for 8x TRN2 NeuronCores (Bass/Tile, data-parallel over batch).

Pipeline per core (8 batch rows):
  gather emb rows (indirect DMA) -> token norms (ACT square+accum) ->
  class dots (DVE fused mul+reduce vs broadcast cls_n) -> s = dot * rsqrt(n2) ->
  s transposed to DRAM (padded) -> im2col X windows -> conv as 2 PE matmuls
  (token-major out [128 l, 2]) -> bias+relu+max (DVE) -> softmax over L
  (PE transpose/ones-matmul for cross-partition max/sum) ->
  beta out + z = exp-weighted emb sum (PE) -> linear head + log_softmax.

Self-contained: hardcodes shapes B=64, L=2048, D=300, C=2, K=111, PAD=55, V=50000.
"""
import numpy as np

B, L, D, C, K, PAD, V = 64, 2048, 300, 2, 111, 55, 50000
P = 128
NCORES = 8
RPC = B // NCORES          # rows per core = 8
NT = L // P                # 16 token-tiles per row
LEXT = L + 2 * PAD         # 2158
F32 = None  # set in build()

_cache = {}


def _build():
    import os
    STAGE = int(os.environ.get("KSTAGE", "6"))
    import concourse.bass as bass
    import concourse.bacc as bacc
    import concourse.tile as tile
    from concourse import mybir
    from concourse.masks import make_identity

    f32 = mybir.dt.float32
    i32 = mybir.dt.int32
    AF = mybir.ActivationFunctionType
    OP = mybir.AluOpType
    AX = mybir.AxisListType

    nc = bacc.Bacc("TRN2", target_bir_lowering=False, debug=False,
                   enable_asserts=False, num_devices=NCORES)

    table = nc.dram_tensor("table", [V, D], f32, kind="ExternalInput").ap()
    idxT = nc.dram_tensor("idxT", [RPC, P, NT], i32, kind="ExternalInput").ap()
    cls = nc.dram_tensor("cls", [C, D], f32, kind="ExternalInput").ap()
    convw = nc.dram_tensor("convw", [C, C, K], f32, kind="ExternalInput").ap()
    convb = nc.dram_tensor("convb", [C], f32, kind="ExternalInput").ap()
    linw = nc.dram_tensor("linw", [C, D], f32, kind="ExternalInput").ap()
    linb = nc.dram_tensor("linb", [C], f32, kind="ExternalInput").ap()
    logits_p = nc.dram_tensor("logits_p", [RPC, C], f32, kind="ExternalOutput").ap()
    beta_p = nc.dram_tensor("beta_p", [RPC, L], f32, kind="ExternalOutput").ap()

    with tile.TileContext(nc) as tc:
        with tc.tile_pool(name="const", bufs=1) as cpool, \
             tc.tile_pool(name="work", bufs=1) as wpool, \
             tc.tile_pool(name="emb", bufs=2) as epool, \
             tc.tile_pool(name="scr", bufs=4) as spool, \
             tc.tile_pool(name="xw", bufs=2) as xpool, \
             tc.tile_pool(name="sdram", bufs=2, space="DRAM") as dpool, \
             tc.tile_pool(name="ps_g", bufs=2, space="PSUM") as gpsum, \
             tc.tile_pool(name="ps_z", bufs=2, space="PSUM") as zpsum, \
             tc.tile_pool(name="ps_tp", bufs=2, space="PSUM") as tpsum, \
             tc.tile_pool(name="ps_st", bufs=2, space="PSUM") as stpsum:

            # ---------- one-time setup ----------
            ident = cpool.tile([P, P], f32)
            make_identity(nc, ident[:])
            ones1 = cpool.tile([1, P], f32)
            nc.gpsimd.memset(ones1[:], 1.0)
            ones_col = cpool.tile([P, 1], f32)
            nc.gpsimd.memset(ones_col[:], 1.0)
            zero55 = cpool.tile([C, PAD], f32)
            nc.gpsimd.memset(zero55[:], 0.0)

            cls_bc = []
            for c in range(C):
                cls_row = cpool.tile([1, D], f32, name=f"cls_row{c}")
                nc.sync.dma_start(out=cls_row[:], in_=cls[c:c+1, :])
                cls_sq = wpool.tile([1, D], f32, tag="clssq", bufs=2)
                cn2 = cpool.tile([1, 1], f32, name=f"cn2_{c}")
                nc.scalar.activation(out=cls_sq[:], in_=cls_row[:], func=AF.Square,
                                     accum_out=cn2[:])
                cnorm = cpool.tile([1, 1], f32, name=f"cnorm{c}")
                nc.scalar.activation(out=cnorm[:], in_=cn2[:], func=AF.Sqrt)
                rcn = cpool.tile([1, 1], f32, name=f"rcn{c}")
                nc.vector.reciprocal(rcn[:], cnorm[:])
                cls_n_c = cpool.tile([1, D], f32, name=f"cls_n{c}")
                nc.vector.tensor_scalar_mul(cls_n_c[:], cls_row[:], rcn[:])
                pbc = stpsum.tile([P, D], f32, tag="st")
                nc.tensor.matmul(out=pbc[:], lhsT=ones1[:], rhs=cls_n_c[:],
                                 start=True, stop=True)
                sb = cpool.tile([P, D], f32, name=f"cls_bc{c}")
                nc.vector.tensor_copy(out=sb[:], in_=pbc[:])
                cls_bc.append(sb)

            Wt = []
            for c1 in range(C):
                w = cpool.tile([K, C], f32, name=f"Wt{c1}")
                nc.sync.dma_start(out=w[:], in_=convw[:, c1, :].rearrange("a k -> k a"))
                Wt.append(w)

            convb_sb = cpool.tile([1, C], f32)
            nc.sync.dma_start(out=convb_sb[:], in_=convb[None, :])
            pcb = stpsum.tile([P, C], f32, tag="st")
            nc.tensor.matmul(out=pcb[:], lhsT=ones1[:], rhs=convb_sb[:],
                             start=True, stop=True)
            convb_bc = cpool.tile([P, C], f32)
            nc.vector.tensor_copy(out=convb_bc[:], in_=pcb[:])

            lw_rows = []
            for c in range(C):
                lw_row = cpool.tile([1, D], f32, name=f"lw_row{c}")
                nc.sync.dma_start(out=lw_row[:], in_=linw[c:c+1, :])
                lw_rows.append(lw_row)
            linb_sb = cpool.tile([1, C], f32)
            nc.sync.dma_start(out=linb_sb[:], in_=linb[None, :])

            z_rows = [cpool.tile([1, D], f32, name=f"z_row{r}") for r in range(RPC)]

            # ---------- per-row pipeline ----------
            for r in range(RPC):
                idx_sb = spool.tile([P, NT], i32, tag="idx", bufs=2)
                nc.sync.dma_start(out=idx_sb[:], in_=idxT[r])

                emb = epool.tile([P, NT * D], f32, tag="emb")
                for lt in range(NT):
                    nc.gpsimd.indirect_dma_start(
                        out=emb[:, lt * D:(lt + 1) * D], out_offset=None,
                        in_=table[:],
                        in_offset=bass.IndirectOffsetOnAxis(ap=idx_sb[:, lt:lt+1], axis=0))

                n2 = spool.tile([P, NT], f32, tag="n2", bufs=2)
                for lt in range(NT):
                    sq = spool.tile([P, D], f32, tag="sq")
                    nc.scalar.activation(out=sq[:], in_=emb[:, lt*D:(lt+1)*D],
                                         func=AF.Square, accum_out=n2[:, lt:lt+1])
                if STAGE < 2:
                    continue
                nsq = spool.tile([P, NT], f32, tag="nsq", bufs=2)
                nc.scalar.activation(out=nsq[:], in_=n2[:], func=AF.Sqrt)
                rnorm = spool.tile([P, NT], f32, tag="rn", bufs=2)
                nc.vector.reciprocal(rnorm[:], nsq[:])

                dots = [spool.tile([P, NT], f32, tag=f"dot{c}", bufs=2,
                                   name=f"dots{c}_{r}") for c in range(C)]
                for lt in range(NT):
                    for c in range(C):
                        scr = spool.tile([P, D], f32, tag="dscr")
                        nc.vector.scalar_tensor_tensor(
                            out=scr[:], in0=emb[:, lt*D:(lt+1)*D], scalar=1.0,
                            in1=cls_bc[c][:], op0=OP.mult, op1=OP.mult,
                            accum_out=dots[c][:, lt:lt+1])

                s_row = dpool.tile([C, LEXT], f32, tag="srow")
                nc.sync.dma_start(out=s_row[:, 0:PAD], in_=zero55[:])
                nc.sync.dma_start(out=s_row[:, PAD+L:LEXT], in_=zero55[:])
                for c in range(C):
                    s_t = spool.tile([P, NT], f32, tag=f"s{c}", bufs=2)
                    nc.vector.tensor_tensor(out=s_t[:], in0=dots[c][:], in1=rnorm[:],
                                            op=OP.mult)
                    ps = tpsum.tile([NT, P], f32, tag="tp")
                    nc.tensor.transpose(out=ps[:], in_=s_t[:], identity=ident[:])
                    sT = spool.tile([NT, P], f32, tag=f"sT{c}", bufs=2)
                    nc.vector.tensor_copy(out=sT[:], in_=ps[:])
                    nc.sync.dma_start(
                        out=s_row[c:c+1, PAD:PAD+L].rearrange("o (a b) -> (o a) b", a=NT),
                        in_=sT[:])

                Xs = []
                for c1 in range(C):
                    X = xpool.tile([K, L], f32, tag=f"X{c1}")
                    src = s_row[c1:c1+1, 0:L]
                    src = bass.AP(src.tensor, src.offset, [[1, K], [1, L]])
                    nc.sync.dma_start(out=X[:], in_=src)
                    Xs.append(X)

                if STAGE < 3:
                    continue
                bpt = spool.tile([P, NT], f32, tag="bpt", bufs=2)
                exp_t = spool.tile([P, NT], f32, tag="expt", bufs=2)
                for lt in range(NT):
                    g = gpsum.tile([P, C], f32, tag="g")
                    for c1 in range(C):
                        nc.tensor.matmul(out=g[:], lhsT=Xs[c1][:, lt*P:(lt+1)*P],
                                         rhs=Wt[c1][:], start=(c1 == 0), stop=(c1 == C-1))
                    t1 = spool.tile([P, C], f32, tag="t1")
                    nc.vector.tensor_tensor(out=t1[:], in0=g[:], in1=convb_bc[:],
                                            op=OP.add)
                    t2 = spool.tile([P, 1], f32, tag="t2")
                    nc.vector.tensor_reduce(out=t2[:], in_=t1[:], axis=AX.X, op=OP.max)
                    nc.vector.tensor_scalar_max(bpt[:, lt:lt+1], t2[:], 0.0)

                if STAGE < 4:
                    continue
                # softmax over all L tokens (partitions x NT)
                mx = spool.tile([P, 1], f32, tag="mx", bufs=2)
                nc.vector.tensor_reduce(out=mx[:], in_=bpt[:], axis=AX.X, op=OP.max)
                pmx = tpsum.tile([1, P], f32, tag="tp")
                nc.tensor.transpose(out=pmx[:], in_=mx[:], identity=ident[:])
                mxT = spool.tile([1, P], f32, tag="mxT", bufs=2)
                nc.vector.tensor_copy(out=mxT[:], in_=pmx[:])
                m11 = spool.tile([1, 1], f32, tag="m11", bufs=2)
                nc.vector.tensor_reduce(out=m11[:], in_=mxT[:], axis=AX.X, op=OP.max)
                nm11 = spool.tile([1, 1], f32, tag="nm11", bufs=2)
                nc.vector.tensor_scalar_mul(nm11[:], m11[:], -1.0)
                pnm = stpsum.tile([P, 1], f32, tag="st")
                nc.tensor.matmul(out=pnm[:], lhsT=ones1[:], rhs=nm11[:],
                                 start=True, stop=True)
                nm_bc = spool.tile([P, 1], f32, tag="nmbc_sb", bufs=2)
                nc.vector.tensor_copy(out=nm_bc[:], in_=pnm[:])
                pse = spool.tile([P, 1], f32, tag="pse", bufs=2)
                nc.scalar.activation(out=exp_t[:], in_=bpt[:], func=AF.Exp,
                                     bias=nm_bc[:], accum_out=pse[:])
                psum_se = stpsum.tile([1, 1], f32, tag="st")
                nc.tensor.matmul(out=psum_se[:], lhsT=pse[:], rhs=ones_col[:],
                                 start=True, stop=True)
                se11 = spool.tile([1, 1], f32, tag="se11", bufs=2)
                nc.vector.tensor_copy(out=se11[:], in_=psum_se[:])
                rs11 = spool.tile([1, 1], f32, tag="rs11", bufs=2)
                nc.vector.reciprocal(rs11[:], se11[:])
                prs = stpsum.tile([P, 1], f32, tag="st")
                nc.tensor.matmul(out=prs[:], lhsT=ones1[:], rhs=rs11[:],
                                 start=True, stop=True)
                rs_bc = spool.tile([P, 1], f32, tag="rsbc_sb", bufs=2)
                nc.vector.tensor_copy(out=rs_bc[:], in_=prs[:])

                if STAGE < 5:
                    continue
                beta_t = spool.tile([P, NT], f32, tag="betat", bufs=2)
                nc.vector.tensor_scalar_mul(beta_t[:], exp_t[:], rs_bc[:])
                pbT = tpsum.tile([NT, P], f32, tag="tp")
                nc.tensor.transpose(out=pbT[:], in_=beta_t[:], identity=ident[:])
                bT = spool.tile([NT, P], f32, tag="bT", bufs=2)
                nc.vector.tensor_copy(out=bT[:], in_=pbT[:])
                nc.sync.dma_start(
                    out=beta_p[r:r+1, :].rearrange("o (a b) -> (o a) b", a=NT),
                    in_=bT[:])

                zp = zpsum.tile([1, D], f32, tag="z")
                for lt in range(NT):
                    nc.tensor.matmul(out=zp[:], lhsT=exp_t[:, lt:lt+1],
                                     rhs=emb[:, lt*D:(lt+1)*D],
                                     start=(lt == 0), stop=(lt == NT - 1))
                nc.vector.tensor_scalar_mul(z_rows[r][:], zp[:], rs11[:])

            # ---------- head (per row; engines need partition base 0) ----------
            for r in range(RPC if STAGE >= 6 else 0):
                o = wpool.tile([1, C], f32, tag="o", bufs=2)
                for c in range(C):
                    hscr = wpool.tile([1, D], f32, tag="hscr", bufs=2)
                    nc.vector.scalar_tensor_tensor(
                        out=hscr[:], in0=z_rows[r][:], scalar=1.0,
                        in1=lw_rows[c][:], op0=OP.mult, op1=OP.mult,
                        accum_out=o[:, c:c+1])
                o2 = wpool.tile([1, C], f32, tag="o2", bufs=2)
                nc.vector.tensor_tensor(out=o2[:], in0=o[:], in1=linb_sb[:], op=OP.add)
                m2 = wpool.tile([1, 1], f32, tag="m2", bufs=2)
                nc.vector.tensor_reduce(out=m2[:], in_=o2[:], axis=AX.X, op=OP.max)
                t8 = wpool.tile([1, C], f32, tag="t8", bufs=2)
                nc.vector.tensor_scalar_sub(t8[:], o2[:], m2[:])
                e8 = wpool.tile([1, C], f32, tag="e8", bufs=2)
                se8 = wpool.tile([1, 1], f32, tag="se8", bufs=2)
                nc.scalar.activation(out=e8[:], in_=t8[:], func=AF.Exp, accum_out=se8[:])
                lse = wpool.tile([1, 1], f32, tag="lse", bufs=2)
                nc.scalar.activation(out=lse[:], in_=se8[:], func=AF.Ln)
                lg = wpool.tile([1, C], f32, tag="lg", bufs=2)
                nc.vector.tensor_scalar_sub(lg[:], t8[:], lse[:])
                nc.sync.dma_start(out=logits_p[r:r+1, :], in_=lg[:])

    nc.compile()
    return nc


def _get_nc():
    if "nc" not in _cache:
        _cache["nc"] = _build()
    return _cache["nc"]


def kernel(inputs, emb_table, class_emb, conv_w, conv_b, lin_w, lin_b):
    import concourse.bass_utils as bass_utils

    nc = _get_nc()
    idx = np.asarray(inputs).astype(np.int32)
    table = np.ascontiguousarray(np.asarray(emb_table, dtype=np.float32))
    cls = np.ascontiguousarray(np.asarray(class_emb, dtype=np.float32))
    cw = np.ascontiguousarray(np.asarray(conv_w, dtype=np.float32))
    cb = np.ascontiguousarray(np.asarray(conv_b, dtype=np.float32))
    lw = np.ascontiguousarray(np.asarray(lin_w, dtype=np.float32))
    lb = np.ascontiguousarray(np.asarray(lin_b, dtype=np.float32))

    # idxT[r, p, lt] = idx[row, lt*128 + p]
    idxT = idx.reshape(B, NT, P).transpose(0, 2, 1)

    in_maps = []
    for c in range(NCORES):
        in_maps.append({
            "table": table,
            "idxT": np.ascontiguousarray(idxT[c*RPC:(c+1)*RPC]),
            "cls": cls, "convw": cw, "convb": cb, "linw": lw, "linb": lb,
        })
    res = bass_utils.run_bass_kernel_spmd(nc, in_maps, core_ids=list(range(NCORES)))
    logits = np.concatenate([r["logits_p"] for r in res.results], axis=0)
    beta = np.concatenate([r["beta_p"] for r in res.results], axis=0)[:, :, None]
    return logits, beta
